# revision 1
# baseline (speedup 1.0000x reference)
"""Trainium2 Bass kernel for a 2-layer directed GCN (PyG GCNConv semantics).

Strategy (8-core SPMD, 1D node sharding):
  - Nodes sharded across 8 cores (12500 each, padded to 12544 = 98*128).
  - Edges partitioned by destination core; per core grouped by destination
    tile (128 nodes), sub-grouped by source chunk (int16 gather reach), and
    padded to whole 128-edge tiles with a cross-core-uniform tile count so a
    single SPMD program serves all cores.
  - Per-edge coefficients nrm = dinv[row]*ew*dinv[col] and self-loop weights
    dinv^2 are host-precomputed (graph-structure preprocessing, cacheable).
  - Linearity trick: aggregate raw features first, apply W afterwards:
        out[c] = (sum_e nrm_e * x[row_e] + dinv_c^2 * x_c) @ W + b
    so layer 1 gathers raw x (replicated bf16 table input, no collective);
    one AllGather of the bf16 relu output builds the layer-2 table.
  - Per 128-edge tile on device: batched gpsimd dma_gather (bf16 rows),
    scaled one-hot S[e,d] = (iota==col)*nrm via one fused DVE tensor_scalar
    (bf16, 2x mode), TensorE bf16 matmul S^T @ msgs accumulating the
    destination tile in fp32 PSUM. Supertiles of 4 destination tiles keep 4
    PSUM aggregation banks live across the 4 source-chunk passes
    (+2 transpose +2 output banks = 8).
  - Per destination tile tail (fp32): self term (DVE scalar_tensor_tensor),
    PE transpose, x W (PE), bias via K=1 rank-1 matmul, relu/copy evict on
    the scalar engine. Layer-1 output is stored once, as bf16: it is both
    the AllGather payload for the layer-2 gather table and the layer-2
    self-term source.
"""

import ml_dtypes
import numpy as np

import concourse.bacc as bacc
import concourse.mybir as mybir
import concourse.tile as tile
from concourse.bass_utils import run_bass_kernel_spmd
from concourse.library_config import mlp

N_NODES = 100000
D = 128
N_CORES = 8
NPC = N_NODES // N_CORES          # 12500 nodes per core
TPC = (NPC + 127) // 128          # 98 destination tiles per core
PAD_NPC = TPC * 128               # 12544 padded nodes per core
N_PAD = N_CORES * PAD_NPC         # 100352 padded table rows
NCHUNK = 4
# Asymmetric chunks (each < 32768 for int16 gather reach): cell means land
# safely below 128-tile ceiling boundaries instead of 2 edges under one.
CHUNK_SIZES = (27776, 27392, 28416, 16768)
CHUNK_LO = (0, 27776, 55168, 83584)
assert sum(CHUNK_SIZES) == N_PAD
SUPER = 4                         # dest tiles per supertile (PSUM: 4+2+2)

F32 = mybir.dt.float32
BF16 = mybir.dt.bfloat16
I16 = mybir.dt.int16
NPBF = ml_dtypes.bfloat16


def _build_nc(t_counts):
    """t_counts[chunk][dtile] = edge tiles (uniform across cores)."""
    NSUP = (TPC + SUPER - 1) // SUPER
    NT = int(sum(t_counts[c][d] for c in range(NCHUNK) for d in range(TPC)))
    nc = bacc.Bacc("TRN2", target_bir_lowering=False)

    x_slab = nc.dram_tensor("x_slab", [PAD_NPC, D], F32, kind="ExternalInput")
    x_tab = nc.dram_tensor("x_tab", [N_PAD, D], BF16, kind="ExternalInput")
    gix = nc.dram_tensor("gix", [128, NT * 8], I16, kind="ExternalInput")
    colw = nc.dram_tensor("colw", [128, NT], F32, kind="ExternalInput")
    nrmw = nc.dram_tensor("nrmw", [128, NT], F32, kind="ExternalInput")
    selfw = nc.dram_tensor("selfw", [128, TPC], F32, kind="ExternalInput")
    w1 = nc.dram_tensor("w1", [D, D], F32, kind="ExternalInput")
    b1 = nc.dram_tensor("b1", [1, D], F32, kind="ExternalInput")
    w2 = nc.dram_tensor("w2", [D, D], F32, kind="ExternalInput")
    b2 = nc.dram_tensor("b2", [1, D], F32, kind="ExternalInput")
    ident = nc.dram_tensor("ident", [128, 128], F32, kind="ExternalInput")
    iota = nc.dram_tensor("iota", [128, 128], BF16, kind="ExternalInput")
    ones = nc.dram_tensor("ones", [1, D], F32, kind="ExternalInput")
    out_slab = nc.dram_tensor("out_slab", [NPC, D], F32, kind="ExternalOutput")

    seg_tiles = [[sum(t_counts[c][d]
                      for d in range(s * SUPER, min((s + 1) * SUPER, TPC)))
                  for c in range(NCHUNK)] for s in range(NSUP)]
    max_seg = max(max(row) for row in seg_tiles) or 1

    with tile.TileContext(nc) as tc:
        nc.gpsimd.load_library(mlp)
        with (
            tc.tile_pool(name="const", bufs=1) as constp,
            tc.tile_pool(name="gbuf", bufs=4) as gbufp,
            tc.tile_pool(name="sbld", bufs=8) as sbldp,
            tc.tile_pool(name="tailp", bufs=6) as tailp,
            tc.tile_pool(name="psag", bufs=SUPER, space="PSUM") as psagp,
            tc.tile_pool(name="pst", bufs=2, space="PSUM") as pstp,
            tc.tile_pool(name="pso", bufs=2, space="PSUM") as psop,
            tc.tile_pool(name="dram", bufs=1, space="DRAM") as dramp,
        ):
            gix_s = constp.tile([128, NT * 8], I16, tag="gix")
            colw_s = constp.tile([128, NT], F32, tag="colw")
            nrmw_s = constp.tile([128, NT], F32, tag="nrmw")
            selfw_s = constp.tile([128, TPC], F32, tag="selfw")
            w1_s = constp.tile([D, D], F32, tag="w1")
            b1_s = constp.tile([1, D], F32, tag="b1")
            w2_s = constp.tile([D, D], F32, tag="w2")
            b2_s = constp.tile([1, D], F32, tag="b2")
            id_s = constp.tile([128, 128], F32, tag="ident")
            iota_s = constp.tile([128, 128], BF16, tag="iota")
            ones_s = constp.tile([1, D], F32, tag="ones")
            for dst, srct in ((gix_s, gix), (colw_s, colw), (nrmw_s, nrmw),
                              (selfw_s, selfw), (w1_s, w1), (b1_s, b1),
                              (w2_s, w2), (b2_s, b2), (id_s, ident),
                              (iota_s, iota), (ones_s, ones)):
                nc.sync.dma_start(dst[:], srct[:])

            h_loc_bf = dramp.tile([PAD_NPC, D], BF16, tag="h_loc_bf")
            h_full = dramp.tile([N_PAD, D], BF16, tag="h_full",
                                addr_space="Shared")

            def layer(table, self_src, w_s, b_s, relu, store):
                t0 = 0
                for s in range(NSUP):
                    dlist = list(range(s * SUPER, min((s + 1) * SUPER, TPC)))
                    total_d = {d: sum(t_counts[c][d] for c in range(NCHUNK))
                               for d in dlist}
                    ps_agg = {d: psagp.tile([128, 128], F32, tag="psag",
                                            name=f"psag_{s}_{d}")
                              for d in dlist if total_d[d] > 0}
                    done_d = {d: 0 for d in dlist}
                    for c in range(NCHUNK):
                        seg = seg_tiles[s][c]
                        if seg == 0:
                            continue
                        gt = gbufp.tile([128, max_seg, 128], BF16, tag="gbuf")
                        nc.gpsimd.dma_gather(
                            gt[:, :seg, :],
                            table[CHUNK_LO[c]:CHUNK_LO[c] + CHUNK_SIZES[c], :],
                            gix_s[:, t0 * 8:(t0 + seg) * 8],
                            seg * 128, seg * 128, D,
                            single_packet=False)
                        tt = t0
                        for d in dlist:
                            for _ in range(t_counts[c][d]):
                                s_t = sbldp.tile([128, 128], BF16, tag="sbld")
                                nc.vector.tensor_scalar(
                                    s_t[:], iota_s[:],
                                    colw_s[:, tt:tt + 1], nrmw_s[:, tt:tt + 1],
                                    mybir.AluOpType.is_equal,
                                    mybir.AluOpType.mult)
                                nc.tensor.matmul(
                                    ps_agg[d][:], s_t[:], gt[:, tt - t0, :],
                                    start=(done_d[d] == 0),
                                    stop=(done_d[d] == total_d[d] - 1))
                                done_d[d] += 1
                                tt += 1
                        t0 += seg
                    for d in dlist:
                        xsel = tailp.tile([128, 128],
                                          F32 if self_src is x_slab else BF16,
                                          tag="xsel")
                        nc.sync.dma_start(
                            xsel[:], self_src[d * 128:(d + 1) * 128, :])
                        agg_sb = tailp.tile([128, 128], F32, tag="aggsb")
                        if total_d[d] > 0:
                            nc.vector.scalar_tensor_tensor(
                                agg_sb[:], xsel[:], selfw_s[:, d:d + 1],
                                ps_agg[d][:],
                                mybir.AluOpType.mult, mybir.AluOpType.add)
                        else:
                            nc.vector.tensor_scalar(
                                agg_sb[:], xsel[:], selfw_s[:, d:d + 1], None,
                                mybir.AluOpType.mult)
                        ps_t = pstp.tile([128, 128], F32, tag="pst")
                        nc.tensor.transpose(ps_t[:], agg_sb[:], id_s[:])
                        agg_t = tailp.tile([128, 128], F32, tag="aggt")
                        nc.scalar.activation(
                            agg_t[:], ps_t[:],
                            mybir.ActivationFunctionType.Copy)
                        ps_o = psop.tile([128, 128], F32, tag="pso")
                        nc.tensor.matmul(ps_o[:], agg_t[:], w_s[:],
                                         start=True, stop=False)
                        nc.tensor.matmul(ps_o[:], ones_s[:], b_s[:],
                                         start=False, stop=True)
                        o_sb = tailp.tile([128, 128],
                                          BF16 if relu else F32, tag="osb")
                        func = (mybir.ActivationFunctionType.Relu if relu
                                else mybir.ActivationFunctionType.Copy)
                        nc.scalar.activation(o_sb[:], ps_o[:], func)
                        store(d, ps_o, o_sb)

            def store1(d, ps_o, o_sb):
                nc.sync.dma_start(h_loc_bf[d * 128:(d + 1) * 128, :], o_sb[:])

            def store2(d, ps_o, o_sb):
                lo = d * 128
                hi = min(lo + 128, NPC)
                nc.sync.dma_start(out_slab[lo:hi, :], o_sb[:hi - lo, :])

            layer(x_tab, x_slab, w1_s, b1_s, True, store1)
            nc.gpsimd.collective_compute(
                "AllGather", mybir.AluOpType.bypass,
                replica_groups=[list(range(N_CORES))],
                ins=[h_loc_bf.opt()], outs=[h_full.opt()])
            layer(h_full, h_loc_bf, w2_s, b2_s, False, store2)

    nc.compile()
    return nc


def _preprocess(x, edge_index, edge_weight):
    """Host-side graph preprocessing -> uniform structure + per-core inputs."""
    row = np.asarray(edge_index[0], dtype=np.int64)
    col = np.asarray(edge_index[1], dtype=np.int64)
    ew = np.asarray(edge_weight, dtype=np.float32)
    n_nodes = N_NODES

    deg = np.bincount(col, weights=ew.astype(np.float64), minlength=n_nodes)
    deg = (deg + 1.0).astype(np.float32)
    dinv = (1.0 / np.sqrt(deg)).astype(np.float32)
    nrm = (dinv[row] * ew * dinv[col]).astype(np.float32)
    selfw_n = (dinv * dinv).astype(np.float32)

    core = col // NPC
    dtile = (col - core * NPC) >> 7
    src_core = row // NPC
    pad_row = (src_core * PAD_NPC + (row - src_core * NPC)).astype(np.int64)
    chunk = np.searchsorted(np.asarray(CHUNK_LO), pad_row, side="right") - 1
    sup = dtile // SUPER

    key = ((core * ((TPC + SUPER - 1) // SUPER) + sup) * NCHUNK + chunk) * TPC + dtile
    order = np.argsort(key, kind="stable")
    kcd = (core * NCHUNK + chunk) * TPC + dtile
    counts = np.bincount(kcd, minlength=N_CORES * NCHUNK * TPC)
    counts = counts.reshape(N_CORES, NCHUNK, TPC)
    t_counts = -(-counts.max(axis=0) // 128)       # [NCHUNK, TPC], may be 0

    NSUP = (TPC + SUPER - 1) // SUPER
    slot_base = np.zeros((NCHUNK, TPC), np.int64)
    acc = 0
    for s in range(NSUP):
        for c in range(NCHUNK):
            for d in range(s * SUPER, min((s + 1) * SUPER, TPC)):
                slot_base[c, d] = acc
                acc += int(t_counts[c, d])
    NT = int(acc)

    key_s = key[order]
    group_start = np.concatenate(
        [[0], np.cumsum(np.bincount(key_s, minlength=key.max() + 1))[:-1]])
    rank = np.arange(len(key_s)) - group_start[key_s]

    gixf = np.zeros((N_CORES, NT * 128), np.int16)
    colwf = np.zeros((N_CORES, NT * 128), np.float32)
    nrmwf = np.zeros((N_CORES, NT * 128), np.float32)
    pos = slot_base[chunk[order], dtile[order]] * 128 + rank
    cidx = core[order]
    gixf[cidx, pos] = (pad_row[order]
                       - np.asarray(CHUNK_LO)[chunk[order]]).astype(np.int16)
    colwf[cidx, pos] = ((col - core * NPC)[order] & 127).astype(np.float32)
    nrmwf[cidx, pos] = nrm[order]

    gixw = gixf.reshape(N_CORES, NT * 8, 16).transpose(0, 2, 1)
    gixw = np.ascontiguousarray(np.tile(gixw, (1, 8, 1)))        # [C,128,NT*8]
    colw = np.ascontiguousarray(
        colwf.reshape(N_CORES, NT, 128).transpose(0, 2, 1))
    nrmw = np.ascontiguousarray(
        nrmwf.reshape(N_CORES, NT, 128).transpose(0, 2, 1))

    selfw_pad = np.zeros(N_CORES * PAD_NPC, np.float32)
    idx_all = np.arange(n_nodes)
    c_all = idx_all // NPC
    selfw_pad[c_all * PAD_NPC + (idx_all - c_all * NPC)] = selfw_n
    selfw = np.ascontiguousarray(
        selfw_pad.reshape(N_CORES, TPC, 128).transpose(0, 2, 1))

    x = np.asarray(x, dtype=np.float32)
    x_slabs = np.zeros((N_CORES, PAD_NPC, D), np.float32)
    x_slabs[:, :NPC, :] = x.reshape(N_CORES, NPC, D)
    x_tab = np.zeros((N_PAD, D), NPBF)
    x_tab.reshape(N_CORES, PAD_NPC, D)[:, :NPC, :] = \
        x.reshape(N_CORES, NPC, D).astype(NPBF)

    t_key = tuple(tuple(int(v) for v in t_counts[c]) for c in range(NCHUNK))
    return t_key, gixw, colw, nrmw, selfw, x_slabs, x_tab


_NC_CACHE: dict = {}


def kernel(x, edge_index, edge_weight, W1, b1, W2, b2):
    t_key, gixw, colw, nrmw, selfw, x_slabs, x_tab = _preprocess(
        x, edge_index, edge_weight)

    if t_key not in _NC_CACHE:
        _NC_CACHE[t_key] = _build_nc([list(r) for r in t_key])
    nc = _NC_CACHE[t_key]

    w1_np = np.ascontiguousarray(np.asarray(W1, dtype=np.float32))
    w2_np = np.ascontiguousarray(np.asarray(W2, dtype=np.float32))
    b1_np = np.asarray(b1, dtype=np.float32).reshape(1, D)
    b2_np = np.asarray(b2, dtype=np.float32).reshape(1, D)
    ident = np.eye(128, dtype=np.float32)
    iota = np.tile(np.arange(128), (128, 1)).astype(NPBF)
    ones = np.ones((1, D), np.float32)

    in_maps = []
    for c in range(N_CORES):
        in_maps.append({
            "x_slab": x_slabs[c], "x_tab": x_tab, "gix": gixw[c],
            "colw": colw[c], "nrmw": nrmw[c], "selfw": selfw[c],
            "w1": w1_np, "b1": b1_np, "w2": w2_np, "b2": b2_np,
            "ident": ident, "iota": iota, "ones": ones,
        })

    res = run_bass_kernel_spmd(nc, in_maps, core_ids=list(range(N_CORES)))
    out = np.concatenate([res.results[c]["out_slab"] for c in range(N_CORES)],
                         axis=0)
    return out



# revision 13
# speedup vs baseline: 1.1437x; 1.1437x over previous
"""Trainium2 Bass kernel for a 2-layer directed GCN (PyG GCNConv semantics).

Strategy (8-core SPMD, 1D node sharding):
  - Nodes sharded across 8 cores (12500 each, padded to 12544 = 98*128).
  - Per-edge coefficients nrm = dinv[row]*ew*dinv[col] and self-loop weights
    dinv^2 are host-precomputed (graph-structure preprocessing, cacheable).
  - Linearity trick: aggregate raw features first, apply W afterwards:
        out[c] = (sum_e nrm_e * x[row_e] + dinv_c^2 * x_c) @ W + b
  - Transposed aggregation: per 128-edge tile, matmul(psT, lhsT=gt, rhs=S)
    accumulates psT[f, dest] = sum_e gt[e, f] * S[e, dest] in PSUM, where
    S[e, d] = (iota==col_e)*nrm_e is built by one fused DVE tensor_scalar.
    The [f, d] orientation feeds the weight matmul directly as lhsT - no PE
    transpose in the tail. Self-loop term is one more matmul per dest tile:
    lhsT = x tile (dest rows), rhs = diag(selfw) built the same DVE way.
  - Layer 1: edges partitioned by destination core, cells = (chunk, dtile)
    with 4 source windows for int16 gather reach; gathers raw bf16 x from a
    replicated table (no collective).
  - One AllGather of the bf16 relu output h builds the layer-2 remote table.
  - Layer 2 is split by source locality: edges with local sources gather
    from h_loc (no collective dependency) and are processed DURING the
    AllGather, accumulating into an SBUF fp32 accumulator; remote-source
    edges gather from the allgathered table after it lands, accumulating in
    PSUM; the tail merges acc + psum, applies W2/bias, and stores.
  - Batched supertile IO: xsel/self-source loads and h/out stores move 4
    dest tiles per DMA to amortize per-DMA overheads.
"""

import ml_dtypes
import numpy as np

import concourse.bacc as bacc
import concourse.mybir as mybir
import concourse.tile as tile
from concourse.bass_utils import run_bass_kernel_spmd
from concourse.library_config import mlp

N_NODES = 100000
D = 128
N_CORES = 8
NPC = N_NODES // N_CORES          # 12500 nodes per core
TPC = (NPC + 127) // 128          # 98 destination tiles per core
PAD_NPC = TPC * 128               # 12544 padded nodes per core
N_PAD = N_CORES * PAD_NPC         # 100352 padded table rows
NCHUNK = 4
# Asymmetric chunks (each < 32768 for int16 gather reach).
CHUNK_SIZES = (27776, 27392, 28416, 16768)
CHUNK_LO = (0, 27776, 55168, 83584)
assert sum(CHUNK_SIZES) == N_PAD
SUPER = 4                         # dest tiles per supertile
# Split AllGather: rows [0, SPLIT_R) of each slab go in AG_A (issued after
# supertile SPLIT_S-1 of layer 1), the rest in AG_B. The layer-2 remote pass
# is split by source half so pass A overlaps AG_B.
SPLIT_D = 40                      # dtiles per core in half A (10 supertiles)
SPLIT_R = SPLIT_D * 128           # 5120 rows per core
SPLIT_S = 10
NA_ROWS = 8 * SPLIT_R             # 45056 rows in table A
NB_ROWS = N_PAD - NA_ROWS         # 55296 rows in table B
WIN_A = (0, NA_ROWS // 2)         # 2 windows per half table (< 32768 each)
WIN_A_SZ = (NA_ROWS // 2, NA_ROWS - NA_ROWS // 2)
WIN_B = (0, NB_ROWS // 2)
WIN_B_SZ = (NB_ROWS // 2, NB_ROWS - NB_ROWS // 2)

F32 = mybir.dt.float32
BF16 = mybir.dt.bfloat16
I16 = mybir.dt.int16
NPBF = ml_dtypes.bfloat16
NSUP = (TPC + SUPER - 1) // SUPER


def _seg_tiles(t_counts):
    """t_counts[c][d] -> seg[s][c] = tiles in supertile s, chunk c."""
    nch = len(t_counts)
    return [[sum(t_counts[c][d]
                 for d in range(s * SUPER, min((s + 1) * SUPER, TPC)))
             for c in range(nch)] for s in range(NSUP)]


def _build_nc(t1, tRA, tRB, tL):
    """t1: [NCHUNK][TPC]; tRA/tRB: [2][TPC] (remote pass A/B windows);
    tL: [TPC] (layer2-local). All cross-core uniform tile counts."""
    NT1 = sum(sum(r) for r in t1)
    NTRA = sum(sum(r) for r in tRA)
    NTRB = sum(sum(r) for r in tRB)
    NTL = sum(tL)
    nc = bacc.Bacc("TRN2", target_bir_lowering=False)

    x_slab = nc.dram_tensor("x_slab", [PAD_NPC, D], F32, kind="ExternalInput")
    x_tab = nc.dram_tensor("x_tab", [N_PAD, D], BF16, kind="ExternalInput")
    gix1 = nc.dram_tensor("gix1", [128, NT1 * 8], I16, kind="ExternalInput")
    colw1 = nc.dram_tensor("colw1", [128, NT1], F32, kind="ExternalInput")
    nrmw1 = nc.dram_tensor("nrmw1", [128, NT1], F32, kind="ExternalInput")
    gixRA = nc.dram_tensor("gixRA", [128, NTRA * 8], I16, kind="ExternalInput")
    colwRA = nc.dram_tensor("colwRA", [128, NTRA], F32, kind="ExternalInput")
    nrmwRA = nc.dram_tensor("nrmwRA", [128, NTRA], F32, kind="ExternalInput")
    gixRB = nc.dram_tensor("gixRB", [128, NTRB * 8], I16, kind="ExternalInput")
    colwRB = nc.dram_tensor("colwRB", [128, NTRB], F32, kind="ExternalInput")
    nrmwRB = nc.dram_tensor("nrmwRB", [128, NTRB], F32, kind="ExternalInput")
    gixL = nc.dram_tensor("gixL", [128, NTL * 8], I16, kind="ExternalInput")
    colwL = nc.dram_tensor("colwL", [128, NTL], F32, kind="ExternalInput")
    nrmwL = nc.dram_tensor("nrmwL", [128, NTL], F32, kind="ExternalInput")
    selfw = nc.dram_tensor("selfw", [128, TPC], F32, kind="ExternalInput")
    pidx = nc.dram_tensor("pidx", [128, 1], F32, kind="ExternalInput")
    w1 = nc.dram_tensor("w1", [D, D], F32, kind="ExternalInput")
    b1 = nc.dram_tensor("b1", [1, D], F32, kind="ExternalInput")
    w2 = nc.dram_tensor("w2", [D, D], F32, kind="ExternalInput")
    b2 = nc.dram_tensor("b2", [1, D], F32, kind="ExternalInput")
    iota = nc.dram_tensor("iota", [128, 128], BF16, kind="ExternalInput")
    ones = nc.dram_tensor("ones", [1, D], F32, kind="ExternalInput")
    out_slab = nc.dram_tensor("out_slab", [NPC, D], F32, kind="ExternalOutput")

    seg1 = _seg_tiles(t1)
    segRA = _seg_tiles(tRA)
    segRB = _seg_tiles(tRB)
    segL = [sum(tL[d] for d in range(s * SUPER, min((s + 1) * SUPER, TPC)))
            for s in range(NSUP)]
    max_seg1 = max(max(r) for r in seg1) or 1
    max_segRA = max(max(r) for r in segRA) or 1
    max_segRB = max(max(r) for r in segRB) or 1
    max_segR = max(max_segRA, max_segRB)
    max_segL = max(segL) or 1

    with tile.TileContext(nc) as tc:
        nc.gpsimd.load_library(mlp)
        with (
            tc.tile_pool(name="const", bufs=1) as constp,
            tc.tile_pool(name="gbuf", bufs=3) as gbufp,
            tc.tile_pool(name="gbufL", bufs=2) as gbufLp,
            tc.tile_pool(name="sbld", bufs=6) as sbldp,
            tc.tile_pool(name="selb", bufs=2) as selbp,
            tc.tile_pool(name="hselp", bufs=1) as hselp,
            tc.tile_pool(name="accp", bufs=1) as accp,
            tc.tile_pool(name="tailp", bufs=4) as tailp,
            tc.tile_pool(name="obat", bufs=2) as obatp,
            tc.tile_pool(name="psag", bufs=SUPER, space="PSUM") as psagp,
            tc.tile_pool(name="pso", bufs=2, space="PSUM") as psop,
            tc.tile_pool(name="dram", bufs=1, space="DRAM") as dramp,
        ):
            gix1_s = constp.tile([128, NT1 * 8], I16, tag="gix1")
            colw1_s = constp.tile([128, NT1], F32, tag="colw1")
            nrmw1_s = constp.tile([128, NT1], F32, tag="nrmw1")
            gixRA_s = constp.tile([128, NTRA * 8], I16, tag="gixRA")
            colwRA_s = constp.tile([128, NTRA], F32, tag="colwRA")
            nrmwRA_s = constp.tile([128, NTRA], F32, tag="nrmwRA")
            gixRB_s = constp.tile([128, NTRB * 8], I16, tag="gixRB")
            colwRB_s = constp.tile([128, NTRB], F32, tag="colwRB")
            nrmwRB_s = constp.tile([128, NTRB], F32, tag="nrmwRB")
            gixL_s = constp.tile([128, NTL * 8], I16, tag="gixL")
            colwL_s = constp.tile([128, NTL], F32, tag="colwL")
            nrmwL_s = constp.tile([128, NTL], F32, tag="nrmwL")
            selfw_s = constp.tile([128, TPC], F32, tag="selfw")
            pidx_s = constp.tile([128, 1], F32, tag="pidx")
            w1_s = constp.tile([D, D], F32, tag="w1")
            b1_s = constp.tile([1, D], F32, tag="b1")
            w2_s = constp.tile([D, D], F32, tag="w2")
            b2_s = constp.tile([1, D], F32, tag="b2")
            iota_s = constp.tile([128, 128], BF16, tag="iota")
            ones_s = constp.tile([1, D], F32, tag="ones")
            for dst, srct in ((gix1_s, gix1), (colw1_s, colw1),
                              (nrmw1_s, nrmw1), (gixL_s, gixL),
                              (colwL_s, colwL), (nrmwL_s, nrmwL),
                              (selfw_s, selfw), (pidx_s, pidx),
                              (w1_s, w1), (b1_s, b1), (w2_s, w2), (b2_s, b2),
                              (iota_s, iota), (ones_s, ones)):
                nc.sync.dma_start(dst[:], srct[:])
            # remote-table consts are not needed until after the AllGathers;
            # schedule them into the AG_B window so they don't delay layer 1
            with tc.tile_wait_until(0.9):
                for dst, srct in ((gixRA_s, gixRA), (colwRA_s, colwRA),
                                  (nrmwRA_s, nrmwRA), (gixRB_s, gixRB),
                                  (colwRB_s, colwRB), (nrmwRB_s, nrmwRB)):
                    nc.scalar.dma_start(dst[:], srct[:])

            h_loc_bf = dramp.tile([PAD_NPC, D], BF16, tag="h_loc_bf")
            h_fullA = dramp.tile([NA_ROWS, D], BF16, tag="h_fullA",
                                 addr_space="Shared")
            h_fullB = dramp.tile([NB_ROWS, D], BF16, tag="h_fullB",
                                 addr_space="Shared")

            def diag_tile(d, pool, dt):
                """[128,128] diag(selfw[:, d]) via (iota==pidx)*selfw."""
                s_t = pool.tile([128, 128], dt, tag="sbld" if dt == BF16
                                else "sbldf")
                nc.vector.tensor_scalar(
                    s_t[:], iota_s[:], pidx_s[:, 0:1], selfw_s[:, d:d + 1],
                    mybir.AluOpType.is_equal, mybir.AluOpType.mult)
                return s_t

            def one_hot(colw_s, nrmw_s, tt, pool):
                s_t = pool.tile([128, 128], BF16, tag="sbld")
                nc.vector.tensor_scalar(
                    s_t[:], iota_s[:], colw_s[:, tt:tt + 1],
                    nrmw_s[:, tt:tt + 1],
                    mybir.AluOpType.is_equal, mybir.AluOpType.mult)
                return s_t

            def sup_range(s):
                return list(range(s * SUPER, min((s + 1) * SUPER, TPC)))

            # ---------------- layer 1 ----------------
            slot1 = 0            # running tile slot in the L1 tables
            for s in range(NSUP):
                dlist = sup_range(s)
                total_d = {d: sum(t1[c][d] for c in range(NCHUNK))
                           for d in dlist}
                ps = {d: psagp.tile([128, 128], F32, tag="psag",
                                    name=f"ps1_{s}_{d}")
                      for d in dlist}
                done = {d: 0 for d in dlist}
                for c in range(NCHUNK):
                    seg = seg1[s][c]
                    if seg == 0:
                        continue
                    gt = gbufp.tile([128, max_seg1, 128], BF16, tag="gbuf")
                    nc.gpsimd.dma_gather(
                        gt[:, :seg, :],
                        x_tab[CHUNK_LO[c]:CHUNK_LO[c] + CHUNK_SIZES[c], :],
                        gix1_s[:, slot1 * 8:(slot1 + seg) * 8],
                        seg * 128, seg * 128, D,
                        single_packet=False)
                    tt = slot1
                    for d in dlist:
                        for _ in range(t1[c][d]):
                            s_t = one_hot(colw1_s, nrmw1_s, tt, sbldp)
                            nc.tensor.matmul(
                                ps[d][:], gt[:, tt - slot1, :], s_t[:],
                                start=(done[d] == 0), stop=False)
                            done[d] += 1
                            tt += 1
                    slot1 += seg
                # batched self-source load: x rows for this supertile
                nsd = len(dlist)
                xsel = selbp.tile([128, SUPER, 128], F32, tag="xsel")
                nc.sync.dma_start(
                    xsel[:, :nsd, :].opt(),
                    x_slab[dlist[0] * 128:(dlist[-1] + 1) * 128, :].rearrange(
                        "(n p) d -> p n d", p=128))
                ob = obatp.tile([128, SUPER * 128], BF16, tag="obat")
                for j, d in enumerate(dlist):
                    dg = diag_tile(d, sbldp, F32)
                    nc.tensor.matmul(ps[d][:], xsel[:, j, :], dg[:],
                                     start=(total_d[d] == 0), stop=True)
                    aggT = tailp.tile([128, 128], F32, tag="aggT")
                    nc.scalar.activation(
                        aggT[:], ps[d][:], mybir.ActivationFunctionType.Copy)
                    ps_o = psop.tile([128, 128], F32, tag="pso")
                    nc.tensor.matmul(ps_o[:], aggT[:], w1_s[:],
                                     start=True, stop=False)
                    nc.tensor.matmul(ps_o[:], ones_s[:], b1_s[:],
                                     start=False, stop=True)
                    nc.scalar.activation(ob[:, j * 128:(j + 1) * 128],
                                         ps_o[:],
                                         mybir.ActivationFunctionType.Relu)
                nc.sync.dma_start(
                    h_loc_bf[dlist[0] * 128:(dlist[-1] + 1) * 128, :]
                    .rearrange("(n p) d -> p n d", p=128),
                    ob[:, :nsd * 128].rearrange("p (n d) -> p n d", d=128))
                if s == SPLIT_S - 1:
                    # first-half AllGather: overlaps the rest of layer 1
                    nc.gpsimd.collective_compute(
                        "AllGather", mybir.AluOpType.bypass,
                        replica_groups=[list(range(N_CORES))],
                        ins=[h_loc_bf[0:SPLIT_R, :].opt()],
                        outs=[h_fullA.opt()])

            # ---------------- layer 2: local-source pass -------------------
            # Runs during the AllGathers (depends only on h_loc); scheduled
            # into the AG_B window so it doesn't consume layer-1 bandwidth.
            accT = {}
            hsel = {}
            slotL = 0
            wait_ctx = tc.tile_wait_until(0.9)
            wait_ctx.__enter__()
            for s in range(NSUP):
                dlist = sup_range(s)
                seg = segL[s]
                psl = {}
                if seg > 0:
                    gt = gbufLp.tile([128, max_segL, 128], BF16, tag="gbufL")
                    nc.gpsimd.dma_gather(
                        gt[:, :seg, :], h_loc_bf[:, :],
                        gixL_s[:, slotL * 8:(slotL + seg) * 8],
                        seg * 128, seg * 128, D,
                        single_packet=False)
                    tt = slotL
                    for d in dlist:
                        if tL[d] == 0:
                            continue
                        psl[d] = psagp.tile([128, 128], F32, tag="psag",
                                            name=f"psL_{s}_{d}")
                        for k in range(tL[d]):
                            s_t = one_hot(colwL_s, nrmwL_s, tt, sbldp)
                            nc.tensor.matmul(
                                psl[d][:], gt[:, tt - slotL, :], s_t[:],
                                start=(k == 0), stop=(k == tL[d] - 1))
                            tt += 1
                    slotL += seg
                # batched self-source rows of h (kept resident for the tail)
                nsd = len(dlist)
                hs = hselp.tile([128, SUPER, 128], BF16, tag=f"hsel{s}",
                                name=f"hsel_{s}")
                nc.sync.dma_start(
                    hs[:, :nsd, :].opt(),
                    h_loc_bf[dlist[0] * 128:(dlist[-1] + 1) * 128, :]
                    .rearrange("(n p) d -> p n d", p=128))
                for j, d in enumerate(dlist):
                    hsel[d] = (hs, j)
                    if d in psl:
                        a = accp.tile([128, 128], F32, tag=f"accT{d}",
                                      name=f"accT_{d}")
                        nc.scalar.activation(
                            a[:], psl[d][:],
                            mybir.ActivationFunctionType.Copy)
                        accT[d] = a
            wait_ctx.__exit__(None, None, None)

            # ---------------- second-half AllGather -------------------------
            nc.gpsimd.collective_compute(
                "AllGather", mybir.AluOpType.bypass,
                replica_groups=[list(range(N_CORES))],
                ins=[h_loc_bf[SPLIT_R:PAD_NPC, :].opt()],
                outs=[h_fullB.opt()])

            # ---------------- layer 2 remote pass A (during AG_B) -----------
            # tile_wait_until: schedule-time ordering so the scheduler keeps
            # this pass after the local pass and before pass B.
            wait_ctx = tc.tile_wait_until(1.0)
            wait_ctx.__enter__()
            slotR = 0
            for s in range(NSUP):
                dlist = sup_range(s)
                total_d = {d: sum(tRA[c][d] for c in range(2)) for d in dlist}
                ps = {d: psagp.tile([128, 128], F32, tag="psag",
                                    name=f"psRA_{s}_{d}")
                      for d in dlist if total_d[d] > 0}
                done = {d: 0 for d in dlist}
                for c in range(2):
                    seg = segRA[s][c]
                    if seg == 0:
                        continue
                    gt = gbufp.tile([128, max_segR, 128], BF16, tag="gbuf")
                    nc.gpsimd.dma_gather(
                        gt[:, :seg, :],
                        h_fullA[WIN_A[c]:WIN_A[c] + WIN_A_SZ[c], :],
                        gixRA_s[:, slotR * 8:(slotR + seg) * 8],
                        seg * 128, seg * 128, D,
                        single_packet=False)
                    tt = slotR
                    for d in dlist:
                        for _ in range(tRA[c][d]):
                            s_t = one_hot(colwRA_s, nrmwRA_s, tt, sbldp)
                            nc.tensor.matmul(
                                ps[d][:], gt[:, tt - slotR, :], s_t[:],
                                start=(done[d] == 0),
                                stop=(done[d] == total_d[d] - 1))
                            done[d] += 1
                            tt += 1
                    slotR += seg
                for d in dlist:
                    if d not in ps:
                        continue
                    if d in accT:
                        nc.vector.tensor_tensor(
                            accT[d][:], accT[d][:], ps[d][:],
                            mybir.AluOpType.add)
                    else:
                        a = accp.tile([128, 128], F32, tag=f"accT{d}",
                                      name=f"accT_{d}")
                        nc.scalar.activation(
                            a[:], ps[d][:], mybir.ActivationFunctionType.Copy)
                        accT[d] = a
            wait_ctx.__exit__(None, None, None)

            # ---------------- layer 2 remote pass B + tail ------------------
            wait_ctx = tc.tile_wait_until(2.0)
            wait_ctx.__enter__()
            slotR = 0
            for s in range(NSUP):
                dlist = sup_range(s)
                total_d = {d: sum(tRB[c][d] for c in range(2)) for d in dlist}
                ps = {d: psagp.tile([128, 128], F32, tag="psag",
                                    name=f"psRB_{s}_{d}")
                      for d in dlist}
                done = {d: 0 for d in dlist}
                for c in range(2):
                    seg = segRB[s][c]
                    if seg == 0:
                        continue
                    gt = gbufp.tile([128, max_segR, 128], BF16, tag="gbuf")
                    nc.gpsimd.dma_gather(
                        gt[:, :seg, :],
                        h_fullB[WIN_B[c]:WIN_B[c] + WIN_B_SZ[c], :],
                        gixRB_s[:, slotR * 8:(slotR + seg) * 8],
                        seg * 128, seg * 128, D,
                        single_packet=False)
                    tt = slotR
                    for d in dlist:
                        for _ in range(tRB[c][d]):
                            s_t = one_hot(colwRB_s, nrmwRB_s, tt, sbldp)
                            nc.tensor.matmul(
                                ps[d][:], gt[:, tt - slotR, :], s_t[:],
                                start=(done[d] == 0), stop=False)
                            done[d] += 1
                            tt += 1
                    slotR += seg
                ob = obatp.tile([128, SUPER * 128], F32, tag="obat2")
                nsd = len(dlist)
                for j, d in enumerate(dlist):
                    hs, hj = hsel[d]
                    dg = diag_tile(d, sbldp, BF16)
                    nc.tensor.matmul(ps[d][:], hs[:, hj, :], dg[:],
                                     start=(total_d[d] == 0), stop=True)
                    aggT = tailp.tile([128, 128], F32, tag="aggT")
                    if d in accT:
                        nc.vector.tensor_tensor(
                            aggT[:], accT[d][:], ps[d][:],
                            mybir.AluOpType.add)
                    else:
                        nc.scalar.activation(
                            aggT[:], ps[d][:],
                            mybir.ActivationFunctionType.Copy)
                    ps_o = psop.tile([128, 128], F32, tag="pso")
                    nc.tensor.matmul(ps_o[:], aggT[:], w2_s[:],
                                     start=True, stop=False)
                    nc.tensor.matmul(ps_o[:], ones_s[:], b2_s[:],
                                     start=False, stop=True)
                    nc.scalar.activation(ob[:, j * 128:(j + 1) * 128],
                                         ps_o[:],
                                         mybir.ActivationFunctionType.Copy)
                lo = dlist[0] * 128
                hi = min((dlist[-1] + 1) * 128, NPC)
                nfull = (hi - lo) // 128
                if nfull > 0:
                    nc.sync.dma_start(
                        out_slab[lo:lo + nfull * 128, :]
                        .rearrange("(n p) d -> p n d", p=128),
                        ob[:, :nfull * 128].rearrange("p (n d) -> p n d",
                                                      d=128))
                rem = (hi - lo) - nfull * 128
                if rem > 0:
                    nc.sync.dma_start(
                        out_slab[lo + nfull * 128:hi, :],
                        ob[:rem, nfull * 128:(nfull + 1) * 128])
            wait_ctx.__exit__(None, None, None)

    nc.compile()
    return nc


def _pack_gix(gixf):
    """[C, NT*128] int16 -> [C, 128, NT*8] wrapped/replicated index layout."""
    C, n = gixf.shape
    NT = n // 128
    g = gixf.reshape(C, NT * 8, 16).transpose(0, 2, 1)
    return np.ascontiguousarray(np.tile(g, (1, 8, 1)))


def _pack_w(wf, dtype=np.float32):
    """[C, NT*128] -> [C, 128, NT]."""
    C, n = wf.shape
    NT = n // 128
    return np.ascontiguousarray(
        wf.reshape(C, NT, 128).transpose(0, 2, 1).astype(dtype))


def _cell_tables(sel, core, dtile, key_extra, n_extra, pad_idx, colv, nrmv):
    """Build packed per-core tables for edges selected by `sel`, grouped by
    cells = (key_extra, dtile) laid out in (sup, key_extra, dtile) order.

    Returns (t_counts [n_extra][TPC], gixf, colwf, nrmwf) where the flat
    arrays are [N_CORES, NT*128]."""
    core = core[sel]
    dtile = dtile[sel]
    ke = key_extra[sel]
    pad_idx = pad_idx[sel]
    colv = colv[sel]
    nrmv = nrmv[sel]

    sup = dtile // SUPER
    key = ((core * NSUP + sup) * n_extra + ke) * TPC + dtile
    order = np.argsort(key, kind="stable")
    kcd = (core * n_extra + ke) * TPC + dtile
    counts = np.bincount(kcd, minlength=N_CORES * n_extra * TPC)
    counts = counts.reshape(N_CORES, n_extra, TPC)
    t_counts = -(-counts.max(axis=0) // 128)       # [n_extra, TPC]

    slot_base = np.zeros((n_extra, TPC), np.int64)
    acc = 0
    for s in range(NSUP):
        for c in range(n_extra):
            for d in range(s * SUPER, min((s + 1) * SUPER, TPC)):
                slot_base[c, d] = acc
                acc += int(t_counts[c, d])
    NT = int(acc)

    key_s = key[order]
    group_start = np.concatenate(
        [[0], np.cumsum(np.bincount(key_s, minlength=key.max() + 1))[:-1]])
    rank = np.arange(len(key_s)) - group_start[key_s]

    gixf = np.zeros((N_CORES, NT * 128), np.int16)
    colwf = np.zeros((N_CORES, NT * 128), np.float32)
    nrmwf = np.zeros((N_CORES, NT * 128), np.float32)
    pos = slot_base[ke[order], dtile[order]] * 128 + rank
    cidx = core[order]
    gixf[cidx, pos] = pad_idx[order].astype(np.int16)
    colwf[cidx, pos] = colv[order]
    nrmwf[cidx, pos] = nrmv[order]
    return t_counts, gixf, colwf, nrmwf


def _preprocess(x, edge_index, edge_weight):
    """Host-side graph preprocessing -> uniform structure + per-core inputs."""
    row = np.asarray(edge_index[0], dtype=np.int64)
    col = np.asarray(edge_index[1], dtype=np.int64)
    ew = np.asarray(edge_weight, dtype=np.float32)
    n_nodes = N_NODES

    deg = np.bincount(col, weights=ew.astype(np.float64), minlength=n_nodes)
    deg = (deg + 1.0).astype(np.float32)
    dinv = (1.0 / np.sqrt(deg)).astype(np.float32)
    nrm = (dinv[row] * ew * dinv[col]).astype(np.float32)
    selfw_n = (dinv * dinv).astype(np.float32)

    core = col // NPC
    dtile = (col - core * NPC) >> 7
    colv = ((col - core * NPC) & 127).astype(np.float32)
    src_core = row // NPC
    pad_row = (src_core * PAD_NPC + (row - src_core * NPC)).astype(np.int64)
    chunk = np.searchsorted(np.asarray(CHUNK_LO), pad_row, side="right") - 1
    chunk_rel = pad_row - np.asarray(CHUNK_LO)[chunk]

    # layer 1: all edges, cells = (chunk, dtile)
    all_sel = np.ones(len(row), bool)
    t1, gix1, colw1, nrmw1 = _cell_tables(
        all_sel, core, dtile, chunk, NCHUNK, chunk_rel, colv, nrm)

    # layer 2: local (src on same core) vs remote. Take only floor(min_core
    # count / 128) full tiles of local edges per dtile (zero padding - every
    # local tile is full on every core); the overflow joins the remote pass,
    # which reads the allgathered table that contains the own slab too.
    loc = src_core == core
    loc_idx = row - src_core * NPC          # row within h_loc [0, NPC)
    cd = core * TPC + dtile
    loc_cnt = np.bincount(cd[loc], minlength=N_CORES * TPC)
    loc_cnt = loc_cnt.reshape(N_CORES, TPC)
    tL = (loc_cnt.min(axis=0) // 128).astype(np.int64)     # [TPC]
    order_l = np.argsort(cd[loc], kind="stable")
    li = np.nonzero(loc)[0][order_l]
    grp = np.concatenate(
        [[0], np.cumsum(np.bincount(cd[loc], minlength=N_CORES * TPC))[:-1]])
    rank_l = np.arange(loc.sum()) - grp[cd[li]]
    take = np.zeros(len(row), bool)
    take[li] = rank_l < (tL * 128)[dtile[li]]
    tLm, gixL, colwL, nrmwL = _cell_tables(
        take, core, dtile, np.zeros(len(row), np.int64), 1,
        loc_idx, colv, nrm)
    assert np.array_equal(tLm[0], tL)

    # remote edges split by source half: half A = rows [0, SPLIT_R) of each
    # slab (gathered from h_fullA after the first AllGather), half B = rest.
    rel = row - src_core * NPC               # local row within source slab
    in_a = rel < SPLIT_R
    rowA = src_core * SPLIT_R + rel          # row within table A
    rowB = src_core * (PAD_NPC - SPLIT_R) + (rel - SPLIT_R)
    winA = (rowA >= WIN_A[1]).astype(np.int64)
    relA = rowA - np.asarray(WIN_A)[winA]
    winB = (rowB >= WIN_B[1]).astype(np.int64)
    relB = rowB - np.asarray(WIN_B)[winB]
    tRA, gixRA, colwRA, nrmwRA = _cell_tables(
        (~take) & in_a, core, dtile, winA, 2, relA, colv, nrm)
    tRB, gixRB, colwRB, nrmwRB = _cell_tables(
        (~take) & ~in_a, core, dtile, winB, 2, relB, colv, nrm)

    selfw_pad = np.zeros(N_CORES * PAD_NPC, np.float32)
    idx_all = np.arange(n_nodes)
    c_all = idx_all // NPC
    selfw_pad[c_all * PAD_NPC + (idx_all - c_all * NPC)] = selfw_n
    selfw = np.ascontiguousarray(
        selfw_pad.reshape(N_CORES, TPC, 128).transpose(0, 2, 1))

    x = np.asarray(x, dtype=np.float32)
    x_slabs = np.zeros((N_CORES, PAD_NPC, D), np.float32)
    x_slabs[:, :NPC, :] = x.reshape(N_CORES, NPC, D)
    x_tab = np.zeros((N_PAD, D), NPBF)
    x_tab.reshape(N_CORES, PAD_NPC, D)[:, :NPC, :] = \
        x.reshape(N_CORES, NPC, D).astype(NPBF)

    def tkey(t):
        return tuple(tuple(int(v) for v in r) for r in t)

    struct_key = (tkey(t1), tkey(tRA), tkey(tRB),
                  tuple(int(v) for v in tL))
    tabs = dict(
        gix1=_pack_gix(gix1), colw1=_pack_w(colw1), nrmw1=_pack_w(nrmw1),
        gixRA=_pack_gix(gixRA), colwRA=_pack_w(colwRA),
        nrmwRA=_pack_w(nrmwRA),
        gixRB=_pack_gix(gixRB), colwRB=_pack_w(colwRB),
        nrmwRB=_pack_w(nrmwRB),
        gixL=_pack_gix(gixL), colwL=_pack_w(colwL), nrmwL=_pack_w(nrmwL),
        selfw=selfw, x_slab=x_slabs, x_tab=x_tab)
    return struct_key, tabs


_NC_CACHE: dict = {}


def kernel(x, edge_index, edge_weight, W1, b1, W2, b2):
    struct_key, tabs = _preprocess(x, edge_index, edge_weight)

    if struct_key not in _NC_CACHE:
        t1 = [list(r) for r in struct_key[0]]
        tRA = [list(r) for r in struct_key[1]]
        tRB = [list(r) for r in struct_key[2]]
        tL = list(struct_key[3])
        _NC_CACHE[struct_key] = _build_nc(t1, tRA, tRB, tL)
    nc = _NC_CACHE[struct_key]

    w1_np = np.ascontiguousarray(np.asarray(W1, dtype=np.float32))
    w2_np = np.ascontiguousarray(np.asarray(W2, dtype=np.float32))
    b1_np = np.asarray(b1, dtype=np.float32).reshape(1, D)
    b2_np = np.asarray(b2, dtype=np.float32).reshape(1, D)
    iota = np.tile(np.arange(128), (128, 1)).astype(NPBF)
    pidx = np.arange(128, dtype=np.float32).reshape(128, 1)
    ones = np.ones((1, D), np.float32)

    in_maps = []
    for c in range(N_CORES):
        m = {
            "x_slab": tabs["x_slab"][c], "x_tab": tabs["x_tab"],
            "selfw": tabs["selfw"][c], "pidx": pidx,
            "w1": w1_np, "b1": b1_np, "w2": w2_np, "b2": b2_np,
            "iota": iota, "ones": ones,
        }
        for k in ("gix1", "colw1", "nrmw1", "gixRA", "colwRA", "nrmwRA",
                  "gixRB", "colwRB", "nrmwRB", "gixL", "colwL", "nrmwL"):
            m[k] = tabs[k][c]
        in_maps.append(m)

    res = run_bass_kernel_spmd(nc, in_maps, core_ids=list(range(N_CORES)))
    out = np.concatenate([res.results[c]["out_slab"] for c in range(N_CORES)],
                         axis=0)
    return out


# revision 18
# speedup vs baseline: 1.1531x; 1.0082x over previous
"""Trainium2 Bass kernel for a 2-layer directed GCN (PyG GCNConv semantics).

Strategy (8-core SPMD, 1D node sharding):
  - Nodes sharded across 8 cores (12500 each, padded to 12544 = 98*128).
  - Per-edge coefficients nrm = dinv[row]*ew*dinv[col] and self-loop weights
    dinv^2 are host-precomputed (graph-structure preprocessing, cacheable).
  - Linearity trick: aggregate raw features first, apply W afterwards:
        out[c] = (sum_e nrm_e * x[row_e] + dinv_c^2 * x_c) @ W + b
  - Transposed aggregation: per 128-edge tile, matmul(psT, lhsT=gt, rhs=S)
    accumulates psT[f, dest] = sum_e gt[e, f] * S[e, dest] in PSUM, where
    S[e, d] = (iota==col_e)*nrm_e is built by one fused DVE tensor_scalar.
    The [f, d] orientation feeds the weight matmul directly as lhsT - no PE
    transpose in the tail. Self-loop term is one more matmul per dest tile:
    lhsT = x tile (dest rows), rhs = diag(selfw) built the same DVE way.
  - Layer 1: edges partitioned by destination core, cells = (chunk, dtile)
    with 4 source windows for int16 gather reach; gathers raw bf16 x from a
    replicated table (no collective).
  - One AllGather of the bf16 relu output h builds the layer-2 remote table.
  - Layer 2 is split by source locality: edges with local sources gather
    from h_loc (no collective dependency) and are processed DURING the
    AllGather, accumulating into an SBUF fp32 accumulator; remote-source
    edges gather from the allgathered table after it lands, accumulating in
    PSUM; the tail merges acc + psum, applies W2/bias, and stores.
  - Batched supertile IO: xsel/self-source loads and h/out stores move 4
    dest tiles per DMA to amortize per-DMA overheads.
"""

import ml_dtypes
import numpy as np

import concourse.bacc as bacc
import concourse.mybir as mybir
import concourse.tile as tile
from concourse.bass_utils import run_bass_kernel_spmd
from concourse.library_config import mlp

N_NODES = 100000
D = 128
N_CORES = 8
NPC = N_NODES // N_CORES          # 12500 nodes per core
TPC = (NPC + 127) // 128          # 98 destination tiles per core
PAD_NPC = TPC * 128               # 12544 padded nodes per core
N_PAD = N_CORES * PAD_NPC         # 100352 padded table rows
NCHUNK = 4
# Asymmetric chunks (each < 32768 for int16 gather reach).
CHUNK_SIZES = (27776, 27392, 28416, 16768)
CHUNK_LO = (0, 27776, 55168, 83584)
assert sum(CHUNK_SIZES) == N_PAD
SUPER = 4                         # dest tiles per supertile
# Split AllGather: rows [0, SPLIT_R) of each slab go in AG_A (issued after
# supertile SPLIT_S-1 of layer 1), the rest in AG_B. The layer-2 remote pass
# is split by source half so pass A overlaps AG_B.
SPLIT_D = 40                      # dtiles per core in half A (10 supertiles)
SPLIT_R = SPLIT_D * 128           # 5120 rows per core
SPLIT_S = 10
NA_ROWS = 8 * SPLIT_R             # 45056 rows in table A
NB_ROWS = N_PAD - NA_ROWS         # 55296 rows in table B
WIN_A = (0, 11008)                # 2 windows per half table (< 32768 each)
WIN_A_SZ = (11008, NA_ROWS - 11008)
WIN_B = (0, NB_ROWS // 2)
WIN_B_SZ = (NB_ROWS // 2, NB_ROWS - NB_ROWS // 2)

F32 = mybir.dt.float32
BF16 = mybir.dt.bfloat16
I16 = mybir.dt.int16
NPBF = ml_dtypes.bfloat16
NSUP = (TPC + SUPER - 1) // SUPER


def _seg_tiles(t_counts):
    """t_counts[c][d] -> seg[s][c] = tiles in supertile s, chunk c."""
    nch = len(t_counts)
    return [[sum(t_counts[c][d]
                 for d in range(s * SUPER, min((s + 1) * SUPER, TPC)))
             for c in range(nch)] for s in range(NSUP)]


def _build_nc(t1, tRA, tRB, tL):
    """t1: [NCHUNK][TPC]; tRA/tRB: [2][TPC] (remote pass A/B windows);
    tL: [TPC] (layer2-local). All cross-core uniform tile counts."""
    NT1 = sum(sum(r) for r in t1)
    NTRA = sum(sum(r) for r in tRA)
    NTRB = sum(sum(r) for r in tRB)
    NTL = sum(tL)
    nc = bacc.Bacc("TRN2", target_bir_lowering=False)

    x_slab = nc.dram_tensor("x_slab", [PAD_NPC, D], F32, kind="ExternalInput")
    x_tab = nc.dram_tensor("x_tab", [N_PAD, D], BF16, kind="ExternalInput")
    gix1 = nc.dram_tensor("gix1", [128, NT1 * 8], I16, kind="ExternalInput")
    colw1 = nc.dram_tensor("colw1", [128, NT1], F32, kind="ExternalInput")
    nrmw1 = nc.dram_tensor("nrmw1", [128, NT1], F32, kind="ExternalInput")
    gixRA = nc.dram_tensor("gixRA", [128, NTRA * 8], I16, kind="ExternalInput")
    colwRA = nc.dram_tensor("colwRA", [128, NTRA], F32, kind="ExternalInput")
    nrmwRA = nc.dram_tensor("nrmwRA", [128, NTRA], F32, kind="ExternalInput")
    gixRB = nc.dram_tensor("gixRB", [128, NTRB * 8], I16, kind="ExternalInput")
    colwRB = nc.dram_tensor("colwRB", [128, NTRB], F32, kind="ExternalInput")
    nrmwRB = nc.dram_tensor("nrmwRB", [128, NTRB], F32, kind="ExternalInput")
    gixL = nc.dram_tensor("gixL", [128, NTL * 8], I16, kind="ExternalInput")
    colwL = nc.dram_tensor("colwL", [128, NTL], F32, kind="ExternalInput")
    nrmwL = nc.dram_tensor("nrmwL", [128, NTL], F32, kind="ExternalInput")
    selfw = nc.dram_tensor("selfw", [128, TPC], F32, kind="ExternalInput")
    pidx = nc.dram_tensor("pidx", [128, 1], F32, kind="ExternalInput")
    w1 = nc.dram_tensor("w1", [D, D], F32, kind="ExternalInput")
    b1 = nc.dram_tensor("b1", [1, D], F32, kind="ExternalInput")
    w2 = nc.dram_tensor("w2", [D, D], F32, kind="ExternalInput")
    b2 = nc.dram_tensor("b2", [1, D], F32, kind="ExternalInput")
    iota = nc.dram_tensor("iota", [128, 128], BF16, kind="ExternalInput")
    ones = nc.dram_tensor("ones", [1, D], F32, kind="ExternalInput")
    out_slab = nc.dram_tensor("out_slab", [NPC, D], F32, kind="ExternalOutput")

    seg1 = _seg_tiles(t1)
    segRA = _seg_tiles(tRA)
    segRB = _seg_tiles(tRB)
    segL = [sum(tL[d] for d in range(s * SUPER, min((s + 1) * SUPER, TPC)))
            for s in range(NSUP)]
    max_seg1 = max(max(r) for r in seg1) or 1
    max_segRA = max(max(r) for r in segRA) or 1
    max_segRB = max(max(r) for r in segRB) or 1
    max_segR = max(max_segRA, max_segRB)
    max_segL = max(segL) or 1

    with tile.TileContext(nc) as tc:
        nc.gpsimd.load_library(mlp)
        with (
            tc.tile_pool(name="const", bufs=1) as constp,
            tc.tile_pool(name="gbuf", bufs=3) as gbufp,
            tc.tile_pool(name="gbufL", bufs=2) as gbufLp,
            tc.tile_pool(name="sbld", bufs=6) as sbldp,
            tc.tile_pool(name="selb", bufs=2) as selbp,
            tc.tile_pool(name="hselp", bufs=1) as hselp,
            tc.tile_pool(name="accp", bufs=1) as accp,
            tc.tile_pool(name="tailp", bufs=4) as tailp,
            tc.tile_pool(name="obat", bufs=2) as obatp,
            tc.tile_pool(name="psag", bufs=SUPER, space="PSUM") as psagp,
            tc.tile_pool(name="pso", bufs=2, space="PSUM") as psop,
            tc.tile_pool(name="dram", bufs=1, space="DRAM") as dramp,
        ):
            gix1_s = constp.tile([128, NT1 * 8], I16, tag="gix1")
            colw1_s = constp.tile([128, NT1], F32, tag="colw1")
            nrmw1_s = constp.tile([128, NT1], F32, tag="nrmw1")
            gixRA_s = constp.tile([128, NTRA * 8], I16, tag="gixRA")
            colwRA_s = constp.tile([128, NTRA], F32, tag="colwRA")
            nrmwRA_s = constp.tile([128, NTRA], F32, tag="nrmwRA")
            gixRB_s = constp.tile([128, NTRB * 8], I16, tag="gixRB")
            colwRB_s = constp.tile([128, NTRB], F32, tag="colwRB")
            nrmwRB_s = constp.tile([128, NTRB], F32, tag="nrmwRB")
            gixL_s = constp.tile([128, NTL * 8], I16, tag="gixL")
            colwL_s = constp.tile([128, NTL], F32, tag="colwL")
            nrmwL_s = constp.tile([128, NTL], F32, tag="nrmwL")
            selfw_s = constp.tile([128, TPC], F32, tag="selfw")
            pidx_s = constp.tile([128, 1], F32, tag="pidx")
            w1_s = constp.tile([D, D], F32, tag="w1")
            b1_s = constp.tile([1, D], F32, tag="b1")
            w2_s = constp.tile([D, D], F32, tag="w2")
            b2_s = constp.tile([1, D], F32, tag="b2")
            iota_s = constp.tile([128, 128], BF16, tag="iota")
            ones_s = constp.tile([1, D], F32, tag="ones")
            q = NT1 // 4
            for lo_t, hi_t in ((0, q), (q, 2 * q), (2 * q, 3 * q),
                               (3 * q, NT1)):
                nc.sync.dma_start(gix1_s[:, lo_t * 8:hi_t * 8],
                                  gix1[:, lo_t * 8:hi_t * 8])
            for dst, srct in ((colw1_s, colw1), (nrmw1_s, nrmw1),
                              (iota_s, iota), (pidx_s, pidx),
                              (selfw_s, selfw), (w1_s, w1), (b1_s, b1),
                              (ones_s, ones)):
                nc.sync.dma_start(dst[:], srct[:])
            with tc.tile_wait_until(0.9):
                for dst, srct in ((gixL_s, gixL), (colwL_s, colwL),
                                  (nrmwL_s, nrmwL), (w2_s, w2),
                                  (b2_s, b2)):
                    nc.sync.dma_start(dst[:], srct[:])
            # remote-table consts are not needed until after the AllGathers;
            # schedule them into the AG_B window so they don't delay layer 1
            with tc.tile_wait_until(0.9):
                for dst, srct in ((gixRA_s, gixRA), (colwRA_s, colwRA),
                                  (nrmwRA_s, nrmwRA), (gixRB_s, gixRB),
                                  (colwRB_s, colwRB), (nrmwRB_s, nrmwRB)):
                    nc.scalar.dma_start(dst[:], srct[:])

            h_loc_bf = dramp.tile([PAD_NPC, D], BF16, tag="h_loc_bf")
            h_fullA = dramp.tile([NA_ROWS, D], BF16, tag="h_fullA",
                                 addr_space="Shared")
            h_fullB = dramp.tile([NB_ROWS, D], BF16, tag="h_fullB",
                                 addr_space="Shared")

            def diag_tile(d, pool, dt):
                """[128,128] diag(selfw[:, d]) via (iota==pidx)*selfw."""
                s_t = pool.tile([128, 128], dt, tag="sbld" if dt == BF16
                                else "sbldf")
                nc.vector.tensor_scalar(
                    s_t[:], iota_s[:], pidx_s[:, 0:1], selfw_s[:, d:d + 1],
                    mybir.AluOpType.is_equal, mybir.AluOpType.mult)
                return s_t

            def one_hot(colw_s, nrmw_s, tt, pool):
                s_t = pool.tile([128, 128], BF16, tag="sbld")
                nc.vector.tensor_scalar(
                    s_t[:], iota_s[:], colw_s[:, tt:tt + 1],
                    nrmw_s[:, tt:tt + 1],
                    mybir.AluOpType.is_equal, mybir.AluOpType.mult)
                return s_t

            def sup_range(s):
                return list(range(s * SUPER, min((s + 1) * SUPER, TPC)))

            # ---------------- layer 1 ----------------
            slot1 = 0            # running tile slot in the L1 tables
            for s in range(NSUP):
                dlist = sup_range(s)
                total_d = {d: sum(t1[c][d] for c in range(NCHUNK))
                           for d in dlist}
                ps = {d: psagp.tile([128, 128], F32, tag="psag",
                                    name=f"ps1_{s}_{d}")
                      for d in dlist}
                done = {d: 0 for d in dlist}
                for c in range(NCHUNK):
                    seg = seg1[s][c]
                    if seg == 0:
                        continue
                    gt = gbufp.tile([128, max_seg1, 128], BF16, tag="gbuf")
                    nc.gpsimd.dma_gather(
                        gt[:, :seg, :],
                        x_tab[CHUNK_LO[c]:CHUNK_LO[c] + CHUNK_SIZES[c], :],
                        gix1_s[:, slot1 * 8:(slot1 + seg) * 8],
                        seg * 128, seg * 128, D,
                        single_packet=False)
                    tt = slot1
                    for d in dlist:
                        for _ in range(t1[c][d]):
                            s_t = one_hot(colw1_s, nrmw1_s, tt, sbldp)
                            nc.tensor.matmul(
                                ps[d][:], gt[:, tt - slot1, :], s_t[:],
                                start=(done[d] == 0), stop=False)
                            done[d] += 1
                            tt += 1
                    slot1 += seg
                # batched self-source load: x rows for this supertile
                nsd = len(dlist)
                xsel = selbp.tile([128, SUPER, 128], F32, tag="xsel")
                nc.sync.dma_start(
                    xsel[:, :nsd, :].opt(),
                    x_slab[dlist[0] * 128:(dlist[-1] + 1) * 128, :].rearrange(
                        "(n p) d -> p n d", p=128))
                ob = obatp.tile([128, SUPER * 128], BF16, tag="obat")
                for j, d in enumerate(dlist):
                    dg = diag_tile(d, sbldp, F32)
                    nc.tensor.matmul(ps[d][:], xsel[:, j, :], dg[:],
                                     start=(total_d[d] == 0), stop=True)
                    aggT = tailp.tile([128, 128], F32, tag="aggT")
                    nc.scalar.activation(
                        aggT[:], ps[d][:], mybir.ActivationFunctionType.Copy)
                    ps_o = psop.tile([128, 128], F32, tag="pso")
                    nc.tensor.matmul(ps_o[:], aggT[:], w1_s[:],
                                     start=True, stop=False)
                    nc.tensor.matmul(ps_o[:], ones_s[:], b1_s[:],
                                     start=False, stop=True)
                    nc.scalar.activation(ob[:, j * 128:(j + 1) * 128],
                                         ps_o[:],
                                         mybir.ActivationFunctionType.Relu)
                nc.sync.dma_start(
                    h_loc_bf[dlist[0] * 128:(dlist[-1] + 1) * 128, :]
                    .rearrange("(n p) d -> p n d", p=128),
                    ob[:, :nsd * 128].rearrange("p (n d) -> p n d", d=128))
                if s == SPLIT_S - 1:
                    # first-half AllGather: overlaps the rest of layer 1
                    nc.gpsimd.collective_compute(
                        "AllGather", mybir.AluOpType.bypass,
                        replica_groups=[list(range(N_CORES))],
                        ins=[h_loc_bf[0:SPLIT_R, :].opt()],
                        outs=[h_fullA.opt()])

            # ---------------- layer 2: local-source pass -------------------
            # Runs during the AllGathers (depends only on h_loc); scheduled
            # into the AG_B window so it doesn't consume layer-1 bandwidth.
            accT = {}
            hsel = {}
            slotL = 0
            wait_ctx = tc.tile_wait_until(0.9)
            wait_ctx.__enter__()
            for s in range(NSUP):
                dlist = sup_range(s)
                seg = segL[s]
                psl = {}
                if seg > 0:
                    gt = gbufLp.tile([128, max_segL, 128], BF16, tag="gbufL")
                    nc.gpsimd.dma_gather(
                        gt[:, :seg, :], h_loc_bf[:, :],
                        gixL_s[:, slotL * 8:(slotL + seg) * 8],
                        seg * 128, seg * 128, D,
                        single_packet=False)
                    tt = slotL
                    for d in dlist:
                        if tL[d] == 0:
                            continue
                        psl[d] = psagp.tile([128, 128], F32, tag="psag",
                                            name=f"psL_{s}_{d}")
                        for k in range(tL[d]):
                            s_t = one_hot(colwL_s, nrmwL_s, tt, sbldp)
                            nc.tensor.matmul(
                                psl[d][:], gt[:, tt - slotL, :], s_t[:],
                                start=(k == 0), stop=(k == tL[d] - 1))
                            tt += 1
                    slotL += seg
                # batched self-source rows of h (kept resident for the tail)
                nsd = len(dlist)
                hs = hselp.tile([128, SUPER, 128], BF16, tag=f"hsel{s}",
                                name=f"hsel_{s}")
                nc.sync.dma_start(
                    hs[:, :nsd, :].opt(),
                    h_loc_bf[dlist[0] * 128:(dlist[-1] + 1) * 128, :]
                    .rearrange("(n p) d -> p n d", p=128))
                for j, d in enumerate(dlist):
                    hsel[d] = (hs, j)
                    if d in psl:
                        a = accp.tile([128, 128], F32, tag=f"accT{d}",
                                      name=f"accT_{d}")
                        nc.scalar.activation(
                            a[:], psl[d][:],
                            mybir.ActivationFunctionType.Copy)
                        accT[d] = a
            wait_ctx.__exit__(None, None, None)

            # ---------------- second-half AllGather -------------------------
            nc.gpsimd.collective_compute(
                "AllGather", mybir.AluOpType.bypass,
                replica_groups=[list(range(N_CORES))],
                ins=[h_loc_bf[SPLIT_R:PAD_NPC, :].opt()],
                outs=[h_fullB.opt()])

            # ---------------- layer 2 remote pass A (during AG_B) -----------
            # tile_wait_until: schedule-time ordering so the scheduler keeps
            # this pass after the local pass and before pass B.
            wait_ctx = tc.tile_wait_until(1.0)
            wait_ctx.__enter__()
            slotR = 0
            for s in range(NSUP):
                dlist = sup_range(s)
                total_d = {d: sum(tRA[c][d] for c in range(2)) for d in dlist}
                ps = {d: psagp.tile([128, 128], F32, tag="psag",
                                    name=f"psRA_{s}_{d}")
                      for d in dlist if total_d[d] > 0}
                done = {d: 0 for d in dlist}
                for c in range(2):
                    seg = segRA[s][c]
                    if seg == 0:
                        continue
                    gt = gbufp.tile([128, max_segR, 128], BF16, tag="gbuf")
                    nc.gpsimd.dma_gather(
                        gt[:, :seg, :],
                        h_fullA[WIN_A[c]:WIN_A[c] + WIN_A_SZ[c], :],
                        gixRA_s[:, slotR * 8:(slotR + seg) * 8],
                        seg * 128, seg * 128, D,
                        single_packet=False)
                    tt = slotR
                    for d in dlist:
                        for _ in range(tRA[c][d]):
                            s_t = one_hot(colwRA_s, nrmwRA_s, tt, sbldp)
                            nc.tensor.matmul(
                                ps[d][:], gt[:, tt - slotR, :], s_t[:],
                                start=(done[d] == 0),
                                stop=(done[d] == total_d[d] - 1))
                            done[d] += 1
                            tt += 1
                    slotR += seg
                for d in dlist:
                    if d not in ps:
                        continue
                    if d in accT:
                        nc.vector.tensor_tensor(
                            accT[d][:], accT[d][:], ps[d][:],
                            mybir.AluOpType.add)
                    else:
                        a = accp.tile([128, 128], F32, tag=f"accT{d}",
                                      name=f"accT_{d}")
                        nc.scalar.activation(
                            a[:], ps[d][:], mybir.ActivationFunctionType.Copy)
                        accT[d] = a
            wait_ctx.__exit__(None, None, None)

            # ---------------- layer 2 remote pass B + tail ------------------
            wait_ctx = tc.tile_wait_until(2.0)
            wait_ctx.__enter__()
            slotR = 0
            for s in range(NSUP):
                dlist = sup_range(s)
                total_d = {d: sum(tRB[c][d] for c in range(2)) for d in dlist}
                ps = {d: psagp.tile([128, 128], F32, tag="psag",
                                    name=f"psRB_{s}_{d}")
                      for d in dlist}
                done = {d: 0 for d in dlist}
                for c in range(2):
                    seg = segRB[s][c]
                    if seg == 0:
                        continue
                    gt = gbufp.tile([128, max_segR, 128], BF16, tag="gbuf")
                    nc.gpsimd.dma_gather(
                        gt[:, :seg, :],
                        h_fullB[WIN_B[c]:WIN_B[c] + WIN_B_SZ[c], :],
                        gixRB_s[:, slotR * 8:(slotR + seg) * 8],
                        seg * 128, seg * 128, D,
                        single_packet=False)
                    tt = slotR
                    for d in dlist:
                        for _ in range(tRB[c][d]):
                            s_t = one_hot(colwRB_s, nrmwRB_s, tt, sbldp)
                            nc.tensor.matmul(
                                ps[d][:], gt[:, tt - slotR, :], s_t[:],
                                start=(done[d] == 0), stop=False)
                            done[d] += 1
                            tt += 1
                    slotR += seg
                ob = obatp.tile([128, SUPER * 128], F32, tag="obat2")
                nsd = len(dlist)
                for j, d in enumerate(dlist):
                    hs, hj = hsel[d]
                    dg = diag_tile(d, sbldp, BF16)
                    nc.tensor.matmul(ps[d][:], hs[:, hj, :], dg[:],
                                     start=(total_d[d] == 0), stop=True)
                    aggT = tailp.tile([128, 128], F32, tag="aggT")
                    if d in accT:
                        nc.vector.tensor_tensor(
                            aggT[:], accT[d][:], ps[d][:],
                            mybir.AluOpType.add)
                    else:
                        nc.scalar.activation(
                            aggT[:], ps[d][:],
                            mybir.ActivationFunctionType.Copy)
                    ps_o = psop.tile([128, 128], F32, tag="pso")
                    nc.tensor.matmul(ps_o[:], aggT[:], w2_s[:],
                                     start=True, stop=False)
                    nc.tensor.matmul(ps_o[:], ones_s[:], b2_s[:],
                                     start=False, stop=True)
                    nc.scalar.activation(ob[:, j * 128:(j + 1) * 128],
                                         ps_o[:],
                                         mybir.ActivationFunctionType.Copy)
                lo = dlist[0] * 128
                hi = min((dlist[-1] + 1) * 128, NPC)
                nfull = (hi - lo) // 128
                if nfull > 0:
                    nc.sync.dma_start(
                        out_slab[lo:lo + nfull * 128, :]
                        .rearrange("(n p) d -> p n d", p=128),
                        ob[:, :nfull * 128].rearrange("p (n d) -> p n d",
                                                      d=128))
                rem = (hi - lo) - nfull * 128
                if rem > 0:
                    nc.sync.dma_start(
                        out_slab[lo + nfull * 128:hi, :],
                        ob[:rem, nfull * 128:(nfull + 1) * 128])
            wait_ctx.__exit__(None, None, None)

    nc.compile()
    return nc


def _pack_gix(gixf):
    """[C, NT*128] int16 -> [C, 128, NT*8] wrapped/replicated index layout."""
    C, n = gixf.shape
    NT = n // 128
    g = gixf.reshape(C, NT * 8, 16).transpose(0, 2, 1)
    return np.ascontiguousarray(np.tile(g, (1, 8, 1)))


def _pack_w(wf, dtype=np.float32):
    """[C, NT*128] -> [C, 128, NT]."""
    C, n = wf.shape
    NT = n // 128
    return np.ascontiguousarray(
        wf.reshape(C, NT, 128).transpose(0, 2, 1).astype(dtype))


def _cell_tables(sel, core, dtile, key_extra, n_extra, pad_idx, colv, nrmv):
    """Build packed per-core tables for edges selected by `sel`, grouped by
    cells = (key_extra, dtile) laid out in (sup, key_extra, dtile) order.

    Returns (t_counts [n_extra][TPC], gixf, colwf, nrmwf) where the flat
    arrays are [N_CORES, NT*128]."""
    core = core[sel]
    dtile = dtile[sel]
    ke = key_extra[sel]
    pad_idx = pad_idx[sel]
    colv = colv[sel]
    nrmv = nrmv[sel]

    sup = dtile // SUPER
    key = ((core * NSUP + sup) * n_extra + ke) * TPC + dtile
    order = np.argsort(key, kind="stable")
    kcd = (core * n_extra + ke) * TPC + dtile
    counts = np.bincount(kcd, minlength=N_CORES * n_extra * TPC)
    counts = counts.reshape(N_CORES, n_extra, TPC)
    t_counts = -(-counts.max(axis=0) // 128)       # [n_extra, TPC]

    slot_base = np.zeros((n_extra, TPC), np.int64)
    acc = 0
    for s in range(NSUP):
        for c in range(n_extra):
            for d in range(s * SUPER, min((s + 1) * SUPER, TPC)):
                slot_base[c, d] = acc
                acc += int(t_counts[c, d])
    NT = int(acc)

    key_s = key[order]
    group_start = np.concatenate(
        [[0], np.cumsum(np.bincount(key_s, minlength=key.max() + 1))[:-1]])
    rank = np.arange(len(key_s)) - group_start[key_s]

    gixf = np.zeros((N_CORES, NT * 128), np.int16)
    colwf = np.zeros((N_CORES, NT * 128), np.float32)
    nrmwf = np.zeros((N_CORES, NT * 128), np.float32)
    pos = slot_base[ke[order], dtile[order]] * 128 + rank
    cidx = core[order]
    gixf[cidx, pos] = pad_idx[order].astype(np.int16)
    colwf[cidx, pos] = colv[order]
    nrmwf[cidx, pos] = nrmv[order]
    return t_counts, gixf, colwf, nrmwf


def _preprocess(x, edge_index, edge_weight):
    """Host-side graph preprocessing -> uniform structure + per-core inputs."""
    row = np.asarray(edge_index[0], dtype=np.int64)
    col = np.asarray(edge_index[1], dtype=np.int64)
    ew = np.asarray(edge_weight, dtype=np.float32)
    n_nodes = N_NODES

    deg = np.bincount(col, weights=ew.astype(np.float64), minlength=n_nodes)
    deg = (deg + 1.0).astype(np.float32)
    dinv = (1.0 / np.sqrt(deg)).astype(np.float32)
    nrm = (dinv[row] * ew * dinv[col]).astype(np.float32)
    selfw_n = (dinv * dinv).astype(np.float32)

    core = col // NPC
    dtile = (col - core * NPC) >> 7
    colv = ((col - core * NPC) & 127).astype(np.float32)
    src_core = row // NPC
    pad_row = (src_core * PAD_NPC + (row - src_core * NPC)).astype(np.int64)
    chunk = np.searchsorted(np.asarray(CHUNK_LO), pad_row, side="right") - 1
    chunk_rel = pad_row - np.asarray(CHUNK_LO)[chunk]

    # layer 1: all edges, cells = (chunk, dtile)
    all_sel = np.ones(len(row), bool)
    t1, gix1, colw1, nrmw1 = _cell_tables(
        all_sel, core, dtile, chunk, NCHUNK, chunk_rel, colv, nrm)

    # layer 2: local (src on same core) vs remote. Take only floor(min_core
    # count / 128) full tiles of local edges per dtile (zero padding - every
    # local tile is full on every core); the overflow joins the remote pass,
    # which reads the allgathered table that contains the own slab too.
    loc = src_core == core
    loc_idx = row - src_core * NPC          # row within h_loc [0, NPC)
    cd = core * TPC + dtile
    loc_cnt = np.bincount(cd[loc], minlength=N_CORES * TPC)
    loc_cnt = loc_cnt.reshape(N_CORES, TPC)
    tL = (loc_cnt.min(axis=0) // 128).astype(np.int64)     # [TPC]
    order_l = np.argsort(cd[loc], kind="stable")
    li = np.nonzero(loc)[0][order_l]
    grp = np.concatenate(
        [[0], np.cumsum(np.bincount(cd[loc], minlength=N_CORES * TPC))[:-1]])
    rank_l = np.arange(loc.sum()) - grp[cd[li]]
    take = np.zeros(len(row), bool)
    take[li] = rank_l < (tL * 128)[dtile[li]]
    tLm, gixL, colwL, nrmwL = _cell_tables(
        take, core, dtile, np.zeros(len(row), np.int64), 1,
        loc_idx, colv, nrm)
    assert np.array_equal(tLm[0], tL)

    # remote edges split by source half: half A = rows [0, SPLIT_R) of each
    # slab (gathered from h_fullA after the first AllGather), half B = rest.
    rel = row - src_core * NPC               # local row within source slab
    in_a = rel < SPLIT_R
    rowA = src_core * SPLIT_R + rel          # row within table A
    rowB = src_core * (PAD_NPC - SPLIT_R) + (rel - SPLIT_R)
    winA = (rowA >= WIN_A[1]).astype(np.int64)
    relA = rowA - np.asarray(WIN_A)[winA]
    winB = (rowB >= WIN_B[1]).astype(np.int64)
    relB = rowB - np.asarray(WIN_B)[winB]
    tRA, gixRA, colwRA, nrmwRA = _cell_tables(
        (~take) & in_a, core, dtile, winA, 2, relA, colv, nrm)
    tRB, gixRB, colwRB, nrmwRB = _cell_tables(
        (~take) & ~in_a, core, dtile, winB, 2, relB, colv, nrm)

    selfw_pad = np.zeros(N_CORES * PAD_NPC, np.float32)
    idx_all = np.arange(n_nodes)
    c_all = idx_all // NPC
    selfw_pad[c_all * PAD_NPC + (idx_all - c_all * NPC)] = selfw_n
    selfw = np.ascontiguousarray(
        selfw_pad.reshape(N_CORES, TPC, 128).transpose(0, 2, 1))

    x = np.asarray(x, dtype=np.float32)
    x_slabs = np.zeros((N_CORES, PAD_NPC, D), np.float32)
    x_slabs[:, :NPC, :] = x.reshape(N_CORES, NPC, D)
    x_tab = np.zeros((N_PAD, D), NPBF)
    x_tab.reshape(N_CORES, PAD_NPC, D)[:, :NPC, :] = \
        x.reshape(N_CORES, NPC, D).astype(NPBF)

    def tkey(t):
        return tuple(tuple(int(v) for v in r) for r in t)

    struct_key = (tkey(t1), tkey(tRA), tkey(tRB),
                  tuple(int(v) for v in tL))
    tabs = dict(
        gix1=_pack_gix(gix1), colw1=_pack_w(colw1), nrmw1=_pack_w(nrmw1),
        gixRA=_pack_gix(gixRA), colwRA=_pack_w(colwRA),
        nrmwRA=_pack_w(nrmwRA),
        gixRB=_pack_gix(gixRB), colwRB=_pack_w(colwRB),
        nrmwRB=_pack_w(nrmwRB),
        gixL=_pack_gix(gixL), colwL=_pack_w(colwL), nrmwL=_pack_w(nrmwL),
        selfw=selfw, x_slab=x_slabs, x_tab=x_tab)
    return struct_key, tabs


_NC_CACHE: dict = {}


def kernel(x, edge_index, edge_weight, W1, b1, W2, b2):
    struct_key, tabs = _preprocess(x, edge_index, edge_weight)

    if struct_key not in _NC_CACHE:
        t1 = [list(r) for r in struct_key[0]]
        tRA = [list(r) for r in struct_key[1]]
        tRB = [list(r) for r in struct_key[2]]
        tL = list(struct_key[3])
        _NC_CACHE[struct_key] = _build_nc(t1, tRA, tRB, tL)
    nc = _NC_CACHE[struct_key]

    w1_np = np.ascontiguousarray(np.asarray(W1, dtype=np.float32))
    w2_np = np.ascontiguousarray(np.asarray(W2, dtype=np.float32))
    b1_np = np.asarray(b1, dtype=np.float32).reshape(1, D)
    b2_np = np.asarray(b2, dtype=np.float32).reshape(1, D)
    iota = np.tile(np.arange(128), (128, 1)).astype(NPBF)
    pidx = np.arange(128, dtype=np.float32).reshape(128, 1)
    ones = np.ones((1, D), np.float32)

    in_maps = []
    for c in range(N_CORES):
        m = {
            "x_slab": tabs["x_slab"][c], "x_tab": tabs["x_tab"],
            "selfw": tabs["selfw"][c], "pidx": pidx,
            "w1": w1_np, "b1": b1_np, "w2": w2_np, "b2": b2_np,
            "iota": iota, "ones": ones,
        }
        for k in ("gix1", "colw1", "nrmw1", "gixRA", "colwRA", "nrmwRA",
                  "gixRB", "colwRB", "nrmwRB", "gixL", "colwL", "nrmwL"):
            m[k] = tabs[k][c]
        in_maps.append(m)

    res = run_bass_kernel_spmd(nc, in_maps, core_ids=list(range(N_CORES)))
    out = np.concatenate([res.results[c]["out_slab"] for c in range(N_CORES)],
                         axis=0)
    return out
